# revision 31
# baseline (speedup 1.0000x reference)
"""Trainium2 Bass kernel for nn_Attn2d (3x3 local window attention, 8 heads).

v3: transposed QK-reduce — the qk products are fed through the matmul
STATIONARY port (free in the cost model) with a small [128, 36] reduce
matrix as the moving operand, so each offset's logit reduction costs 36
columns instead of 512. Logits land pixel-major ([128px, 288] psum), so
softmax (exp / z-sum / reciprocal / normalize) runs on ACT+DVE along the
free dim with no PE involvement, and normalization happens BEFORE the AV
stage (po psum holds the final output, copied out directly). A PE
transpose brings normalized attention back to channel-major [100, 512]
for the expand matmuls. All-bf16 (no fp8/DoubleRow).

Layout notes: per-chunk logits psum plT [128, 288] = (h:2, blk:4, n:4,
dl:9); e_t [100, 512] with halves at partitions 0..35 / 64..99 (PE
outputs must start at partition 0/32/64/96)."""
import numpy as np

import concourse.mybir as mybir
import concourse.tile as tile
from concourse import bacc
from concourse.ap import AP

F32 = mybir.dt.float32
F32R = mybir.dt.float32r
BF16 = mybir.dt.bfloat16
AF = mybir.ActivationFunctionType
AX = mybir.AxisListType

B, CIN, H, W = 4, 256, 64, 64
QK = 256
OUT = 256
NH = 8
KW = 3
D = QK // NH
SCALE = float(D) ** (-0.25)
NCORES = 8

HOWN = H // 2
HS = HOWN + 2
WP = W + 4
C0 = 2
PXP = HS * WP
PXU = HS * W
OWNPX = HOWN * W
NKK = KW * KW

CHUNK = 512
NCHUNK = OWNPX // CHUNK
NBLK = CHUNK // 128

PCW = [512, 512, 512, 384, 256]
PCO = [0, 512, 1024, 1536, 1920]

SLOTS = [(0, 1), (2, 3), (4, 5), (6, 7)]
DL8 = 8

# ---- engine config ----
# QK product slots (s, h): 'D' DVE, 'P' Pool (all bf16)
QK_CFG = {(0, 0): 'D', (0, 1): 'P',
          (1, 0): 'P', (1, 1): 'P',
          (2, 0): 'D', (2, 1): 'P',
          (3, 0): 'D', (3, 1): 'P'}
QK8_CFG = {0: 'D', 1: 'D'}
# AV product units (s, h): 'D' DVE direct from psum,
# 'AP' ACT-copy psum->bf16 then Pool multiply
AV_CFG = {(0, 0): 'D', (0, 1): 'AP',
          (1, 0): 'AP', (1, 1): 'AP',
          (2, 0): 'D', (2, 1): 'AP',
          (3, 0): 'AP', (3, 1): 'AP'}
AV8_CFG = {0: 'D', 1: 'AP'}


def _off(dl):
    return (dl // 3) * WP + 1 + (dl % 3)


def _build_nc():
    nc = bacc.Bacc()

    xin = nc.declare_dram_parameter("x", [CIN, PXU], F32R, isOutput=False)
    wtrd = nc.declare_dram_parameter("wtr", [128, 2 * 768], F32R, isOutput=False)
    biasd = nc.declare_dram_parameter("bias", [128, 6], F32, isOutput=False)
    posbd = nc.declare_dram_parameter("posb", [128, 72], BF16, isOutput=False)
    redbd = nc.declare_dram_parameter("redb", [128, NKK * 36], BF16, isOutput=False)
    ematd = nc.declare_dram_parameter("emat2", [100, NKK * 128], BF16, isOutput=False)
    idbd = nc.declare_dram_parameter("idb", [128, 128], BF16, isOutput=False)
    edged = nc.declare_dram_parameter("edge", [128, 2], F32, isOutput=False)
    outd = nc.declare_dram_parameter("o", [OUT, OWNPX], F32, isOutput=True)

    with tile.TileContext(nc) as tc:
        with (
            tc.tile_pool(name="const", bufs=1) as constp,
            tc.tile_pool(name="data", bufs=1) as datap,
            tc.tile_pool(name="qkp", bufs=8) as qkp,
            tc.tile_pool(name="p2p", bufs=8) as p2p,
            tc.tile_pool(name="ebp", bufs=6) as ebp,
            tc.tile_pool(name="psl", bufs=1, space="PSUM") as psl,
            tc.tile_pool(name="pse", bufs=3, space="PSUM") as pse,
            tc.tile_pool(name="pso", bufs=1, space="PSUM") as pso,
        ):
            # PE warmup (p-state ramp)
            wz = constp.tile([1, 2], BF16, tag="wz", name="wz")
            nc.gpsimd.memset(wz[:], 0.0)
            pwz = pse.tile([1, 2], F32, tag="pe", name="pwz")
            nc.tensor.matmul(pwz[:, 0:1], wz[:, 0:1], wz[:, 1:2],
                             start=True, stop=True, skip_group_check=True)

            # ---- loads (queues ordered by first use) ----
            xf = datap.tile([128, 2 * PXU], F32R, tag="xf", name="xf")
            xfv = xf[:].rearrange("p (t x) -> p t x", t=2)

            def xrhs(pci, t):
                co, cw = PCO[pci], PCW[pci]
                return xfv[:, t, co:co + cw]

            def xdma(eng, pci, t):
                co, cw = PCO[pci], PCW[pci]
                eng.dma_start(xfv[:, t, co:co + cw],
                              xin[t * 128:(t + 1) * 128, co:co + cw])

            xdma(nc.sync, 0, 0)
            xdma(nc.sync, 0, 1)
            wt8 = constp.tile([128, 2 * 768], F32R, tag="wt8", name="wt8")
            wt8v = wt8[:].rearrange("p (two m) -> p two m", two=2)
            wtdv = wtrd[:].rearrange("p (two m) -> p two m", two=2)
            for g in range(3):
                nc.gpsimd.dma_start(wt8v[:, :, g * 256:(g + 1) * 256],
                                    wtdv[:, :, g * 256:(g + 1) * 256])
            bias_t = constp.tile([128, 6], F32, tag="bias", name="bias")
            nc.scalar.dma_start(bias_t[:], biasd[:])
            xdma(nc.sync, 1, 0)
            xdma(nc.scalar, 1, 1)
            posb = constp.tile([128, 72], BF16, tag="posb", name="posb")
            nc.gpsimd.dma_start(posb[:], posbd[:])
            redb = constp.tile([128, NKK * 36], BF16, tag="redb", name="redb")
            nc.gpsimd.dma_start(redb[:], redbd[:])
            xdma(nc.sync, 2, 0)
            xdma(nc.scalar, 2, 1)
            idb = constp.tile([128, 128], BF16, tag="idb", name="idb")
            nc.sync.dma_start(idb[:], idbd[:])
            emat2 = constp.tile([100, NKK * 128], BF16, tag="emat2", name="emat2")
            nc.sync.dma_start(emat2[:], ematd[:])
            edge_t = constp.tile([128, 2], F32, tag="edge", name="edge")
            nc.scalar.dma_start(edge_t[:], edged[:])
            xdma(nc.sync, 3, 0)
            xdma(nc.scalar, 3, 1)
            xdma(nc.gpsimd, 4, 0)
            xdma(nc.gpsimd, 4, 1)

            q2 = datap.tile([128, 2 * PXP], BF16, tag="q2", name="q2")
            k2 = datap.tile([128, 2 * PXP], BF16, tag="k2", name="k2")
            v2 = datap.tile([128, 2 * PXP], BF16, tag="v2", name="v2")

            def padv(tl, t, r0, nr, c0, ncol):
                return (tl[:].rearrange("p (t r c) -> p t r c", t=2, c=WP)
                        [:, t, r0:r0 + nr, c0:c0 + ncol])

            for tl in (k2, v2):
                pv = tl[:].rearrange("p (t r c) -> p t r c", t=2, c=WP)
                nc.gpsimd.memset(pv[:, :, :, 0:C0], 0.0)
                nc.gpsimd.memset(pv[:, :, :, WP - C0:WP], 0.0)

            # per-chunk-parity softmax tiles
            eP = [datap.tile([128, 288], BF16, tag=f"eP{i}", name=f"eP{i}")
                  for i in range(2)]
            aP = [datap.tile([128, 288], BF16, tag=f"aP{i}", name=f"aP{i}")
                  for i in range(2)]
            zt = [datap.tile([128, 32], F32, tag=f"zt{i}", name=f"zt{i}")
                  for i in range(2)]
            rz = [datap.tile([128, 32], F32, tag=f"rz{i}", name=f"rz{i}")
                  for i in range(2)]
            et = [datap.tile([100, 512], BF16, tag=f"et{i}", name=f"et{i}")
                  for i in range(2)]

            # logits psum, double-buffered; e_t transpose staging (bf16 psum)
            plT = [psl.tile([128, 288], F32, tag=f"plT{i}", name=f"plT{i}")
                   for i in range(2)]
            etP = psl.tile([100, 512], BF16, tag="etP", name="etP")
            # rows 36..63 of etP are never written by the transposes but are
            # covered by the [100, 512] copy to et -> zero them once
            nc.vector.memset(etP[36:64, :], 0.0)

            # ---- projection (fp32r; x consumed via bitcast, no copy) ----
            def gen_proj(pci):
                for mb0 in range(0, 6, 2):
                    emit_proj(pci, mb0, mb0 + 2)
                    yield

            def emit_proj(pci, mb_lo=0, mb_hi=6):
                co, cw = PCO[pci], PCW[pci]
                r0, nr = co // W, cw // W
                for mb in range(mb_lo, mb_hi):
                    pp = pse.tile([128, cw], F32, tag="pe", name=f"pp{pci}_{mb}")
                    for i in range(2):
                        lhsT = wt8[:][:, i * 768 + mb * 128: i * 768 + (mb + 1) * 128]
                        nc.tensor.matmul(
                            pp[:], lhsT, xrhs(pci, i),
                            start=(i == 0), stop=(i == 1),
                            skip_group_check=True)
                    grp, t = mb // 2, mb % 2
                    dst = (q2, k2, v2)[grp]
                    nc.vector.tensor_scalar_add(padv(dst, t, r0, nr, C0, W),
                                                pp[:], bias_t[:, mb:mb + 1])

            def emit_edge(row, col):
                for tl in (k2, v2):
                    pv = tl[:].rearrange("p (t r c) -> p t r c", t=2, c=WP)
                    nc.gpsimd.tensor_scalar_mul(pv[:, :, row, :], pv[:, :, row, :],
                                                edge_t[:, col:col + 1])

            # ---- views ----
            def kv_pair(tl, h, ci, dla, dlb):
                base = h * PXP + (8 * ci) * WP + _off(dla)
                delta = _off(dlb) - _off(dla)
                return AP(tensor=tl[:].tensor, offset=tl[:].offset + base,
                          ap=[[2 * PXP, 128], [delta, 2], [WP, 8], [1, W]])

            def kv_single(tl, h, ci, dl):
                base = h * PXP + (8 * ci) * WP + _off(dl)
                return AP(tensor=tl[:].tensor, offset=tl[:].offset + base,
                          ap=[[2 * PXP, 128], [WP, 8], [1, W]])

            def q_own(h, ci):
                return padv(q2, h, 1 + 8 * ci, 8, C0, W)

            def q_dup(h, ci):
                return q_own(h, ci).unsqueeze(1).broadcast_to([128, 2, 8, W])

            def q_row(h, ci, r):
                # [128, W] = one image row of q (2D AP for the stationary port)
                return padv(q2, h, 1 + 8 * ci + r, 1, C0, W).squeeze(1)

            def pl_slice(ci, h, b):
                return plT[ci % 2][:, h * 144 + b * 36: h * 144 + (b + 1) * 36]

            def emit_reduce(ci, h, qk_ap, dl, stop):
                # qk_ap: [128, CHUNK] bf16 product (flat view); consumed as
                # stationary in 128-px blocks; moving = redb column block.
                for b in range(NBLK):
                    nc.tensor.matmul(
                        pl_slice(ci, h, b),
                        qk_ap[:, b * 128:(b + 1) * 128],
                        redb[:, dl * 36:(dl + 1) * 36],
                        start=False, stop=stop, skip_group_check=True)

            def gen_red(ci):
                """QK products (DVE/Pool) + transposed reduce (PE tiny MMs)."""
                for h in range(2):
                    for b in range(NBLK):
                        c0 = h * 144 + b * 36
                        for r in range(2):
                            # stationary must be a 2D AP -> one 64-px image
                            # row per pos matmul, output at partition 0/64
                            nc.tensor.matmul(
                                plT[ci % 2][r * 64:(r + 1) * 64, c0:c0 + 36],
                                q_row(h, ci, 2 * b + r),
                                posb[:, h * 36:(h + 1) * 36],
                                start=True, stop=False, skip_group_check=True)
                yield
                for s, (dla, dlb) in enumerate(SLOTS):
                    for h in range(2):
                        eng = nc.gpsimd if QK_CFG[(s, h)] == 'P' else nc.vector
                        qk = qkp.tile([128, 2 * CHUNK], BF16, tag="qk",
                                      name=f"qk{ci}_{s}_{h}")
                        qkv = qk[:].rearrange("p (two r c) -> p two r c", two=2, c=W)
                        with nc.allow_low_precision(reason="qk products"):
                            eng.tensor_mul(qkv, q_dup(h, ci),
                                           kv_pair(k2, h, ci, dla, dlb))
                        qkf = qk[:].rearrange("p (two x) -> p two x", two=2)
                        emit_reduce(ci, h, qkf[:, 0], dla, stop=False)
                        emit_reduce(ci, h, qkf[:, 1], dlb, stop=False)
                    yield
                for h in range(2):
                    eng = nc.gpsimd if QK8_CFG[h] == 'P' else nc.vector
                    qs = qkp.tile([128, CHUNK], BF16, tag="qks",
                                  name=f"qks{ci}_{h}")
                    qsv = qs[:].rearrange("p (r c) -> p r c", c=W)
                    with nc.allow_low_precision(reason="qk products"):
                        eng.tensor_mul(qsv, q_own(h, ci), kv_single(k2, h, ci, DL8))
                    emit_reduce(ci, h, qs[:], DL8, stop=True)
                yield

            def gen_av(ci):
                par = ci % 2
                # softmax: exp -> z -> 1/z -> normalize (pixel-major)
                nc.scalar.activation(eP[par][:], plT[par][:], AF.Exp)
                ev = eP[par][:].rearrange("p (h b n d) -> p h b n d",
                                          h=2, b=NBLK, n=4)
                ztv = zt[par][:].rearrange("p (h b n) -> p h b n", h=2, b=NBLK)
                nc.vector.tensor_reduce(ztv, ev, AX.X, mybir.AluOpType.add)
                nc.vector.reciprocal(rz[par][:], zt[par][:])
                rzv = (rz[par][:].rearrange("p (h b n) -> p h b n", h=2, b=NBLK)
                       .unsqueeze(4).broadcast_to([128, 2, NBLK, 4, NKK]))
                av = aP[par][:].rearrange("p (h b n d) -> p h b n d",
                                          h=2, b=NBLK, n=4)
                with nc.allow_low_precision(reason="normalized attention"):
                    nc.vector.tensor_mul(av, ev, rzv)
                # transpose back to channel-major [100, 512] (halves at 0/64)
                for h in range(2):
                    for b in range(NBLK):
                        nc.tensor.transpose(
                            etP[h * 64: h * 64 + 36, b * 128:(b + 1) * 128],
                            aP[par][:, h * 144 + b * 36: h * 144 + (b + 1) * 36],
                            idb[:])
                nc.vector.tensor_copy(et[par][:], etP[:])
                yield

                def ematv(h, dl):
                    return emat2[h * 64: h * 64 + 36, dl * 128:(dl + 1) * 128]

                def etv(h):
                    return et[par][h * 64: h * 64 + 36, :]

                po = [pso.tile([128, CHUNK], F32, tag=f"po{h}", name=f"po{ci}_{h}")
                      for h in range(2)]
                started = [False, False]

                # h-major: finish half 0 early so its output copy + DMA
                # overlap half 1's units (shrinks the kernel tail)
                units = [(s, h) for h in range(2) for s in range(5)]

                def emit_expands(u):
                    s, h = units[u]
                    if s < 4:
                        dla, dlb = SLOTS[s]
                        ea = pse.tile([128, CHUNK], F32, tag="pe",
                                      name=f"ea{ci}_{s}_{h}")
                        nc.tensor.matmul(ea[:], ematv(h, dla), etv(h),
                                         start=True, stop=True,
                                         skip_group_check=True)
                        eb = pse.tile([128, CHUNK], F32, tag="pe",
                                      name=f"eb{ci}_{s}_{h}")
                        nc.tensor.matmul(eb[:], ematv(h, dlb), etv(h),
                                         start=True, stop=True,
                                         skip_group_check=True)
                        return (ea, eb)
                    ea = pse.tile([128, CHUNK], F32, tag="pe", name=f"ea8{ci}_{h}")
                    nc.tensor.matmul(ea[:], ematv(h, DL8), etv(h),
                                     start=True, stop=True, skip_group_check=True)
                    return (ea,)

                def av_product(dst, src_psum, h, dl, cfg, tagsuffix):
                    if cfg == 'D':
                        with nc.allow_low_precision(reason="av products"):
                            nc.vector.tensor_mul(
                                dst, src_psum[:].rearrange("p (r c) -> p r c", c=W),
                                kv_single(v2, h, ci, dl))
                    else:  # 'AP'
                        cb = ebp.tile([128, CHUNK], BF16, tag="ebf",
                                      name=f"cb{tagsuffix}")
                        nc.scalar.copy(cb[:], src_psum[:])
                        with nc.allow_low_precision(reason="av products"):
                            nc.gpsimd.tensor_mul(
                                dst, cb[:].rearrange("p (r c) -> p r c", c=W),
                                kv_single(v2, h, ci, dl))

                def emit_unit(u, exp_tiles):
                    s, h = units[u]
                    if s < 4:
                        dla, dlb = SLOTS[s]
                        ea, eb = exp_tiles
                        cfg = AV_CFG[(s, h)]
                        p2 = p2p.tile([128, 2 * CHUNK], BF16, tag="p2",
                                      name=f"p2{ci}_{s}_{h}")
                        p2v = p2[:].rearrange("p (two r c) -> p two r c",
                                              two=2, c=W)
                        av_product(p2v[:, 0], ea, h, dla, cfg, f"{ci}_{s}_{h}_a")
                        av_product(p2v[:, 1], eb, h, dlb, cfg, f"{ci}_{s}_{h}_b")
                        for j in range(2):
                            nc.tensor.matmul(
                                po[h][:], idb[:],
                                p2[:].rearrange("p (two x) -> p two x", two=2)[:, j, :],
                                start=not started[h], stop=False,
                                skip_group_check=True)
                            started[h] = True
                    else:
                        (ea,) = exp_tiles
                        cfg = AV8_CFG[h]
                        ps_ = p2p.tile([128, CHUNK], BF16, tag="p2s",
                                       name=f"p2s{ci}_{h}")
                        av_product(ps_[:].rearrange("p (r c) -> p r c", c=W),
                                   ea, h, DL8, cfg, f"8{ci}_{h}")
                        nc.tensor.matmul(po[h][:], idb[:], ps_[:],
                                         start=False, stop=True,
                                         skip_group_check=True)

                def emit_out(h):
                    # output copy + DMA (already normalized)
                    ob = datap.tile([128, CHUNK], F32, tag=f"ob{h}", name=f"ob{ci}_{h}")
                    if h == 0:
                        nc.scalar.copy(ob[:], po[h][:])
                    else:
                        nc.vector.tensor_copy(ob[:], po[h][:])
                    (nc.sync if h == 0 else nc.scalar).dma_start(
                        outd[h * 128:(h + 1) * 128, ci * CHUNK:(ci + 1) * CHUNK],
                        ob[:])

                nxt = emit_expands(0)
                yield
                for u in range(len(units)):
                    cur = nxt
                    if u + 1 < len(units):
                        nxt = emit_expands(u + 1)
                    emit_unit(u, cur)
                    if units[u][0] == 4:
                        emit_out(units[u][1])
                    yield

            def drive(gens, headstart=2):
                live = list(gens)
                for _ in range(headstart):
                    if live:
                        try:
                            next(live[0])
                        except StopIteration:
                            live.pop(0)
                while live:
                    nxt = []
                    for g in live:
                        try:
                            next(g)
                            nxt.append(g)
                        except StopIteration:
                            pass
                    live = nxt

            # ---- schedule ----
            emit_proj(0)
            emit_proj(1)
            emit_edge(0, 0)
            drive([gen_red(0)])
            emit_proj(2)
            drive([gen_av(0), gen_red(1), gen_proj(3)], headstart=2)
            drive([gen_av(1), gen_red(2), gen_proj(4)], headstart=2)
            emit_edge(HS - 1, 1)
            drive([gen_av(2), gen_red(3)], headstart=2)
            drive([gen_av(3)])

    nc.finalize()
    return nc


_CACHE = {}


def _host_consts(w_proj, b_proj, pos_feats):
    import ml_dtypes
    bf = ml_dtypes.bfloat16

    w = np.asarray(w_proj, np.float32).copy()       # [768, 256]
    w[: 2 * QK] *= SCALE
    b = np.asarray(b_proj, np.float32).copy()
    b[: 2 * QK] *= SCALE
    bias = np.ascontiguousarray(b.reshape(6, 128).T)

    wtr = np.zeros((128, 2 * 768), np.float32)
    for i in range(2):
        wtr[:, i * 768:(i + 1) * 768] = w[:, i * 128:(i + 1) * 128].T

    pf = np.asarray(pos_feats, np.float32)          # [256, 9]
    posb = np.zeros((128, 72), np.float32)          # [c2, (h, row36)]
    for c2 in range(128):
        n_loc = c2 // 32
        for dl in range(NKK):
            posb[c2, 0 * 36 + n_loc * 9 + dl] = pf[c2, dl]
            posb[c2, 1 * 36 + n_loc * 9 + dl] = pf[128 + c2, dl]

    redb = np.zeros((128, NKK * 36), np.float32)
    for dl in range(NKK):
        for c2 in range(128):
            redb[c2, dl * 36 + (c2 // 32) * 9 + dl] = 1.0

    emat2 = np.zeros((100, NKK * 128), np.float32)
    for h in range(2):
        for r in range(36):
            n_loc, dl = r // 9, r % 9
            emat2[h * 64 + r,
                  dl * 128 + n_loc * 32: dl * 128 + (n_loc + 1) * 32] = 1.0

    idb = np.eye(128, dtype=np.float32)

    return (wtr.astype(np.float32), bias, posb.astype(bf), redb.astype(bf),
            emat2.astype(bf), idb.astype(bf))


def make_in_maps(x, w_proj, b_proj, pos_feats):
    x = np.asarray(x, np.float32)
    (wtr, bias, posb, redb, emat2, idb) = _host_consts(
        np.asarray(w_proj, np.float32),
        np.asarray(b_proj, np.float32),
        np.asarray(pos_feats, np.float32),
    )
    in_maps = []
    for s in range(NCORES):
        b_i, half = s // 2, s % 2
        xs = np.zeros((CIN, HS, W), np.float32)
        h0 = half * HOWN - 1
        lo, hi = max(h0, 0), min(h0 + HS, H)
        xs[:, lo - h0:hi - h0, :] = x[b_i, :, lo:hi, :]
        edge = np.ones((128, 2), np.float32)
        if half == 0:
            edge[:, 0] = 0.0
        if half == 1:
            edge[:, 1] = 0.0
        in_maps.append({
            "x": np.ascontiguousarray(xs.reshape(CIN, PXU)),
            "wtr": wtr, "bias": bias, "posb": posb, "redb": redb,
            "emat2": emat2, "idb": idb, "edge": edge,
        })
    return in_maps


def kernel(x, w_proj, b_proj, pos_feats):
    from concourse.bass_utils import run_bass_kernel_spmd

    if "nc" not in _CACHE:
        _CACHE["nc"] = _build_nc()
    nc = _CACHE["nc"]
    in_maps = make_in_maps(x, w_proj, b_proj, pos_feats)
    res = run_bass_kernel_spmd(nc, in_maps, list(range(NCORES)))
    out = np.zeros((B, OUT, H, W), np.float32)
    for s in range(NCORES):
        b_i, half = s // 2, s % 2
        out[b_i, :, half * HOWN:(half + 1) * HOWN, :] = (
            res.results[s]["o"].reshape(OUT, HOWN, W)
        )
    return out


# revision 34
# speedup vs baseline: 1.0264x; 1.0264x over previous
"""Trainium2 Bass kernel for nn_Attn2d (3x3 local window attention, 8 heads).

v3: transposed QK-reduce — the qk products are fed through the matmul
STATIONARY port (free in the cost model) with a small [128, 36] reduce
matrix as the moving operand, so each offset's logit reduction costs 36
columns instead of 512. Logits land pixel-major ([128px, 288] psum), so
softmax (exp / z-sum / reciprocal / normalize) runs on ACT+DVE along the
free dim with no PE involvement, and normalization happens BEFORE the AV
stage (po psum holds the final output, copied out directly). A PE
transpose brings normalized attention back to channel-major [100, 512]
for the expand matmuls. All-bf16 (no fp8/DoubleRow).

Layout notes: per-chunk logits psum plT [128, 288] = (h:2, blk:4, n:4,
dl:9); e_t [100, 512] with halves at partitions 0..35 / 64..99 (PE
outputs must start at partition 0/32/64/96)."""
import numpy as np

import concourse.mybir as mybir
import concourse.tile as tile
from concourse import bacc
from concourse.ap import AP

F32 = mybir.dt.float32
F32R = mybir.dt.float32r
BF16 = mybir.dt.bfloat16
AF = mybir.ActivationFunctionType
AX = mybir.AxisListType

B, CIN, H, W = 4, 256, 64, 64
QK = 256
OUT = 256
NH = 8
KW = 3
D = QK // NH
SCALE = float(D) ** (-0.25)
NCORES = 8

HOWN = H // 2
HS = HOWN + 2
WP = W + 4
C0 = 2
PXP = HS * WP
PXU = HS * W
OWNPX = HOWN * W
NKK = KW * KW

CHUNK = 512
NCHUNK = OWNPX // CHUNK
NBLK = CHUNK // 128

PCW = [512, 512, 512, 384, 256]
PCO = [0, 512, 1024, 1536, 1920]

SLOTS = [(0, 1), (2, 3), (4, 5), (6, 7)]
DL8 = 8

# ---- engine config (LP-balanced: ACT=proj+cb+ob, Pool=QK, DVE=AV) ----
# QK product slots (s, h): 'D' DVE, 'P' Pool (all bf16)
QK_CFG = {(0, 0): 'D', (0, 1): 'P',
          (1, 0): 'P', (1, 1): 'P',
          (2, 0): 'P', (2, 1): 'P',
          (3, 0): 'P', (3, 1): 'P'}
QK8_CFG = {0: 'P', 1: 'P'}
# AV product units (s, h): 'D' DVE direct from psum,
# 'AP' ACT-copy psum->bf16 then Pool multiply
AV_CFG = {(0, 0): 'D', (0, 1): 'D',
          (1, 0): 'D', (1, 1): 'D',
          (2, 0): 'D', (2, 1): 'AP',
          (3, 0): 'D', (3, 1): 'AP'}
AV8_CFG = {0: 'D', 1: 'AP'}


def _off(dl):
    return (dl // 3) * WP + 1 + (dl % 3)


def _build_nc():
    nc = bacc.Bacc()

    xin = nc.declare_dram_parameter("x", [CIN, PXU], F32R, isOutput=False)
    wtrd = nc.declare_dram_parameter("wtr", [128, 2 * 768], F32R, isOutput=False)
    biasd = nc.declare_dram_parameter("bias", [128, 6], F32, isOutput=False)
    posbd = nc.declare_dram_parameter("posb", [128, 72], BF16, isOutput=False)
    redbd = nc.declare_dram_parameter("redb", [128, NKK * 36], BF16, isOutput=False)
    ematd = nc.declare_dram_parameter("emat2", [100, NKK * 128], BF16, isOutput=False)
    idbd = nc.declare_dram_parameter("idb", [128, 128], BF16, isOutput=False)
    edged = nc.declare_dram_parameter("edge", [128, 2], F32, isOutput=False)
    outd = nc.declare_dram_parameter("o", [OUT, OWNPX], F32, isOutput=True)

    with tile.TileContext(nc) as tc:
        with (
            tc.tile_pool(name="const", bufs=1) as constp,
            tc.tile_pool(name="data", bufs=1) as datap,
            tc.tile_pool(name="qkp", bufs=8) as qkp,
            tc.tile_pool(name="p2p", bufs=8) as p2p,
            tc.tile_pool(name="ebp", bufs=6) as ebp,
            tc.tile_pool(name="psl", bufs=1, space="PSUM") as psl,
            tc.tile_pool(name="pse", bufs=3, space="PSUM") as pse,
            tc.tile_pool(name="pso", bufs=1, space="PSUM") as pso,
        ):
            # PE warmup (p-state ramp)
            wz = constp.tile([1, 2], BF16, tag="wz", name="wz")
            nc.gpsimd.memset(wz[:], 0.0)
            pwz = pse.tile([1, 2], F32, tag="pe", name="pwz")
            nc.tensor.matmul(pwz[:, 0:1], wz[:, 0:1], wz[:, 1:2],
                             start=True, stop=True, skip_group_check=True)

            # ---- loads (queues ordered by first use) ----
            xf = datap.tile([128, 2 * PXU], F32R, tag="xf", name="xf")
            xfv = xf[:].rearrange("p (t x) -> p t x", t=2)

            def xrhs(pci, t):
                co, cw = PCO[pci], PCW[pci]
                return xfv[:, t, co:co + cw]

            def xdma(eng, pci, t):
                co, cw = PCO[pci], PCW[pci]
                eng.dma_start(xfv[:, t, co:co + cw],
                              xin[t * 128:(t + 1) * 128, co:co + cw])

            xdma(nc.sync, 0, 0)
            xdma(nc.sync, 0, 1)
            wt8 = constp.tile([128, 2 * 768], F32R, tag="wt8", name="wt8")
            wt8v = wt8[:].rearrange("p (two m) -> p two m", two=2)
            wtdv = wtrd[:].rearrange("p (two m) -> p two m", two=2)
            for g in range(3):
                nc.gpsimd.dma_start(wt8v[:, :, g * 256:(g + 1) * 256],
                                    wtdv[:, :, g * 256:(g + 1) * 256])
            bias_t = constp.tile([128, 6], F32, tag="bias", name="bias")
            nc.scalar.dma_start(bias_t[:], biasd[:])
            xdma(nc.sync, 1, 0)
            xdma(nc.scalar, 1, 1)
            posb = constp.tile([128, 72], BF16, tag="posb", name="posb")
            nc.gpsimd.dma_start(posb[:], posbd[:])
            redb = constp.tile([128, NKK * 36], BF16, tag="redb", name="redb")
            nc.gpsimd.dma_start(redb[:], redbd[:])
            xdma(nc.sync, 2, 0)
            xdma(nc.scalar, 2, 1)
            idb = constp.tile([128, 128], BF16, tag="idb", name="idb")
            nc.sync.dma_start(idb[:], idbd[:])
            emat2 = constp.tile([100, NKK * 128], BF16, tag="emat2", name="emat2")
            nc.sync.dma_start(emat2[:], ematd[:])
            edge_t = constp.tile([128, 2], F32, tag="edge", name="edge")
            nc.scalar.dma_start(edge_t[:], edged[:])
            xdma(nc.sync, 3, 0)
            xdma(nc.scalar, 3, 1)
            xdma(nc.gpsimd, 4, 0)
            xdma(nc.gpsimd, 4, 1)

            q2 = datap.tile([128, 2 * PXP], BF16, tag="q2", name="q2")
            k2 = datap.tile([128, 2 * PXP], BF16, tag="k2", name="k2")
            v2 = datap.tile([128, 2 * PXP], BF16, tag="v2", name="v2")

            def padv(tl, t, r0, nr, c0, ncol):
                return (tl[:].rearrange("p (t r c) -> p t r c", t=2, c=WP)
                        [:, t, r0:r0 + nr, c0:c0 + ncol])

            for tl in (k2, v2):
                pv = tl[:].rearrange("p (t r c) -> p t r c", t=2, c=WP)
                nc.gpsimd.memset(pv[:, :, :, 0:C0], 0.0)
                nc.gpsimd.memset(pv[:, :, :, WP - C0:WP], 0.0)

            # per-chunk-parity softmax tiles
            eP = [datap.tile([128, 288], BF16, tag=f"eP{i}", name=f"eP{i}")
                  for i in range(2)]
            aP = [datap.tile([128, 288], BF16, tag=f"aP{i}", name=f"aP{i}")
                  for i in range(2)]
            zt = [datap.tile([128, 32], F32, tag=f"zt{i}", name=f"zt{i}")
                  for i in range(2)]
            rz = [datap.tile([128, 32], F32, tag=f"rz{i}", name=f"rz{i}")
                  for i in range(2)]
            et = [datap.tile([100, 512], BF16, tag=f"et{i}", name=f"et{i}")
                  for i in range(2)]

            # logits psum, double-buffered; e_t transpose staging (bf16 psum)
            plT = [psl.tile([128, 288], F32, tag=f"plT{i}", name=f"plT{i}")
                   for i in range(2)]
            etP = psl.tile([100, 512], BF16, tag="etP", name="etP")
            # rows 36..63 of etP are never written by the transposes but are
            # covered by the [100, 512] copy to et -> zero them once
            nc.vector.memset(etP[36:64, :], 0.0)

            # ---- projection (fp32r; x consumed via bitcast, no copy) ----
            def gen_proj(pci):
                for mb0 in range(0, 6, 2):
                    emit_proj(pci, mb0, mb0 + 2)
                    yield

            def emit_proj(pci, mb_lo=0, mb_hi=6):
                co, cw = PCO[pci], PCW[pci]
                r0, nr = co // W, cw // W
                for mb in range(mb_lo, mb_hi):
                    pp = pse.tile([128, cw], F32, tag="pe", name=f"pp{pci}_{mb}")
                    for i in range(2):
                        lhsT = wt8[:][:, i * 768 + mb * 128: i * 768 + (mb + 1) * 128]
                        nc.tensor.matmul(
                            pp[:], lhsT, xrhs(pci, i),
                            start=(i == 0), stop=(i == 1),
                            skip_group_check=True)
                    grp, t = mb // 2, mb % 2
                    dst = (q2, k2, v2)[grp]
                    nc.scalar.activation(padv(dst, t, r0, nr, C0, W), pp[:],
                                         AF.Identity, bias=bias_t[:, mb:mb + 1])

            def emit_edge(row, col):
                for tl in (k2, v2):
                    pv = tl[:].rearrange("p (t r c) -> p t r c", t=2, c=WP)
                    nc.gpsimd.tensor_scalar_mul(pv[:, :, row, :], pv[:, :, row, :],
                                                edge_t[:, col:col + 1])

            # ---- views ----
            def kv_pair(tl, h, ci, dla, dlb):
                base = h * PXP + (8 * ci) * WP + _off(dla)
                delta = _off(dlb) - _off(dla)
                return AP(tensor=tl[:].tensor, offset=tl[:].offset + base,
                          ap=[[2 * PXP, 128], [delta, 2], [WP, 8], [1, W]])

            def kv_single(tl, h, ci, dl):
                base = h * PXP + (8 * ci) * WP + _off(dl)
                return AP(tensor=tl[:].tensor, offset=tl[:].offset + base,
                          ap=[[2 * PXP, 128], [WP, 8], [1, W]])

            def q_own(h, ci):
                return padv(q2, h, 1 + 8 * ci, 8, C0, W)

            def q_dup(h, ci):
                return q_own(h, ci).unsqueeze(1).broadcast_to([128, 2, 8, W])

            def q_row(h, ci, r):
                # [128, W] = one image row of q (2D AP for the stationary port)
                return padv(q2, h, 1 + 8 * ci + r, 1, C0, W).squeeze(1)

            def pl_slice(ci, h, b):
                return plT[ci % 2][:, h * 144 + b * 36: h * 144 + (b + 1) * 36]

            def emit_reduce(ci, h, qk_ap, dl, stop):
                # qk_ap: [128, CHUNK] bf16 product (flat view); consumed as
                # stationary in 128-px blocks; moving = redb column block.
                for b in range(NBLK):
                    nc.tensor.matmul(
                        pl_slice(ci, h, b),
                        qk_ap[:, b * 128:(b + 1) * 128],
                        redb[:, dl * 36:(dl + 1) * 36],
                        start=False, stop=stop, skip_group_check=True)

            def gen_red(ci):
                """QK products (DVE/Pool) + transposed reduce (PE tiny MMs)."""
                for h in range(2):
                    for b in range(NBLK):
                        c0 = h * 144 + b * 36
                        for r in range(2):
                            # stationary must be a 2D AP -> one 64-px image
                            # row per pos matmul, output at partition 0/64
                            nc.tensor.matmul(
                                plT[ci % 2][r * 64:(r + 1) * 64, c0:c0 + 36],
                                q_row(h, ci, 2 * b + r),
                                posb[:, h * 36:(h + 1) * 36],
                                start=True, stop=False, skip_group_check=True)
                yield
                for s, (dla, dlb) in enumerate(SLOTS):
                    for h in range(2):
                        eng = nc.gpsimd if QK_CFG[(s, h)] == 'P' else nc.vector
                        qk = qkp.tile([128, 2 * CHUNK], BF16, tag="qk",
                                      name=f"qk{ci}_{s}_{h}")
                        qkv = qk[:].rearrange("p (two r c) -> p two r c", two=2, c=W)
                        with nc.allow_low_precision(reason="qk products"):
                            eng.tensor_mul(qkv, q_dup(h, ci),
                                           kv_pair(k2, h, ci, dla, dlb))
                        qkf = qk[:].rearrange("p (two x) -> p two x", two=2)
                        emit_reduce(ci, h, qkf[:, 0], dla, stop=False)
                        emit_reduce(ci, h, qkf[:, 1], dlb, stop=False)
                    yield
                for h in range(2):
                    eng = nc.gpsimd if QK8_CFG[h] == 'P' else nc.vector
                    qs = qkp.tile([128, CHUNK], BF16, tag="qks",
                                  name=f"qks{ci}_{h}")
                    qsv = qs[:].rearrange("p (r c) -> p r c", c=W)
                    with nc.allow_low_precision(reason="qk products"):
                        eng.tensor_mul(qsv, q_own(h, ci), kv_single(k2, h, ci, DL8))
                    emit_reduce(ci, h, qs[:], DL8, stop=True)
                yield

            def gen_av(ci):
                par = ci % 2
                # softmax: exp -> z -> 1/z -> normalize (pixel-major)
                nc.scalar.activation(eP[par][:], plT[par][:], AF.Exp)
                ev = eP[par][:].rearrange("p (h b n d) -> p h b n d",
                                          h=2, b=NBLK, n=4)
                ztv = zt[par][:].rearrange("p (h b n) -> p h b n", h=2, b=NBLK)
                nc.vector.tensor_reduce(ztv, ev, AX.X, mybir.AluOpType.add)
                nc.vector.reciprocal(rz[par][:], zt[par][:])
                rzv = (rz[par][:].rearrange("p (h b n) -> p h b n", h=2, b=NBLK)
                       .unsqueeze(4).broadcast_to([128, 2, NBLK, 4, NKK]))
                av = aP[par][:].rearrange("p (h b n d) -> p h b n d",
                                          h=2, b=NBLK, n=4)
                with nc.allow_low_precision(reason="normalized attention"):
                    nc.vector.tensor_mul(av, ev, rzv)
                # transpose back to channel-major [100, 512] (halves at 0/64)
                for h in range(2):
                    for b in range(NBLK):
                        nc.tensor.transpose(
                            etP[h * 64: h * 64 + 36, b * 128:(b + 1) * 128],
                            aP[par][:, h * 144 + b * 36: h * 144 + (b + 1) * 36],
                            idb[:])
                nc.vector.tensor_copy(et[par][:], etP[:])
                yield

                def ematv(h, dl):
                    return emat2[h * 64: h * 64 + 36, dl * 128:(dl + 1) * 128]

                def etv(h):
                    return et[par][h * 64: h * 64 + 36, :]

                po = [pso.tile([128, CHUNK], F32, tag=f"po{h}", name=f"po{ci}_{h}")
                      for h in range(2)]
                started = [False, False]

                # h-major: finish half 0 early so its output copy + DMA
                # overlap half 1's units (shrinks the kernel tail)
                units = [(s, h) for h in range(2) for s in range(5)]

                def emit_expands(u):
                    s, h = units[u]
                    if s < 4:
                        dla, dlb = SLOTS[s]
                        ea = pse.tile([128, CHUNK], F32, tag="pe",
                                      name=f"ea{ci}_{s}_{h}")
                        nc.tensor.matmul(ea[:], ematv(h, dla), etv(h),
                                         start=True, stop=True,
                                         skip_group_check=True)
                        eb = pse.tile([128, CHUNK], F32, tag="pe",
                                      name=f"eb{ci}_{s}_{h}")
                        nc.tensor.matmul(eb[:], ematv(h, dlb), etv(h),
                                         start=True, stop=True,
                                         skip_group_check=True)
                        return (ea, eb)
                    ea = pse.tile([128, CHUNK], F32, tag="pe", name=f"ea8{ci}_{h}")
                    nc.tensor.matmul(ea[:], ematv(h, DL8), etv(h),
                                     start=True, stop=True, skip_group_check=True)
                    return (ea,)

                def av_product(dst, src_psum, h, dl, cfg, tagsuffix):
                    if cfg == 'D':
                        with nc.allow_low_precision(reason="av products"):
                            nc.vector.tensor_mul(
                                dst, src_psum[:].rearrange("p (r c) -> p r c", c=W),
                                kv_single(v2, h, ci, dl))
                    else:  # 'AP'
                        cb = ebp.tile([128, CHUNK], BF16, tag="ebf",
                                      name=f"cb{tagsuffix}")
                        nc.scalar.copy(cb[:], src_psum[:])
                        with nc.allow_low_precision(reason="av products"):
                            nc.gpsimd.tensor_mul(
                                dst, cb[:].rearrange("p (r c) -> p r c", c=W),
                                kv_single(v2, h, ci, dl))

                def emit_unit(u, exp_tiles):
                    s, h = units[u]
                    if s < 4:
                        dla, dlb = SLOTS[s]
                        ea, eb = exp_tiles
                        cfg = AV_CFG[(s, h)]
                        p2 = p2p.tile([128, 2 * CHUNK], BF16, tag="p2",
                                      name=f"p2{ci}_{s}_{h}")
                        p2v = p2[:].rearrange("p (two r c) -> p two r c",
                                              two=2, c=W)
                        av_product(p2v[:, 0], ea, h, dla, cfg, f"{ci}_{s}_{h}_a")
                        av_product(p2v[:, 1], eb, h, dlb, cfg, f"{ci}_{s}_{h}_b")
                        for j in range(2):
                            nc.tensor.matmul(
                                po[h][:], idb[:],
                                p2[:].rearrange("p (two x) -> p two x", two=2)[:, j, :],
                                start=not started[h], stop=False,
                                skip_group_check=True)
                            started[h] = True
                    else:
                        (ea,) = exp_tiles
                        cfg = AV8_CFG[h]
                        ps_ = p2p.tile([128, CHUNK], BF16, tag="p2s",
                                       name=f"p2s{ci}_{h}")
                        av_product(ps_[:].rearrange("p (r c) -> p r c", c=W),
                                   ea, h, DL8, cfg, f"8{ci}_{h}")
                        nc.tensor.matmul(po[h][:], idb[:], ps_[:],
                                         start=False, stop=True,
                                         skip_group_check=True)

                def emit_out(h):
                    # output copy + DMA (already normalized)
                    ob = datap.tile([128, CHUNK], F32, tag=f"ob{h}", name=f"ob{ci}_{h}")
                    nc.scalar.copy(ob[:], po[h][:])
                    (nc.sync if h == 0 else nc.scalar).dma_start(
                        outd[h * 128:(h + 1) * 128, ci * CHUNK:(ci + 1) * CHUNK],
                        ob[:])

                nxt = emit_expands(0)
                yield
                for u in range(len(units)):
                    cur = nxt
                    if u + 1 < len(units):
                        nxt = emit_expands(u + 1)
                    emit_unit(u, cur)
                    if units[u][0] == 4:
                        emit_out(units[u][1])
                    yield

            def drive(gens, headstart=2):
                live = list(gens)
                for _ in range(headstart):
                    if live:
                        try:
                            next(live[0])
                        except StopIteration:
                            live.pop(0)
                while live:
                    nxt = []
                    for g in live:
                        try:
                            next(g)
                            nxt.append(g)
                        except StopIteration:
                            pass
                    live = nxt

            # ---- schedule ----
            emit_proj(0)
            emit_proj(1)
            emit_edge(0, 0)
            drive([gen_red(0)])
            emit_proj(2)
            drive([gen_av(0), gen_red(1), gen_proj(3)], headstart=2)
            drive([gen_av(1), gen_red(2), gen_proj(4)], headstart=2)
            emit_edge(HS - 1, 1)
            drive([gen_av(2), gen_red(3)], headstart=2)
            drive([gen_av(3)])

    nc.finalize()
    return nc


_CACHE = {}


def _host_consts(w_proj, b_proj, pos_feats):
    import ml_dtypes
    bf = ml_dtypes.bfloat16

    w = np.asarray(w_proj, np.float32).copy()       # [768, 256]
    w[: 2 * QK] *= SCALE
    b = np.asarray(b_proj, np.float32).copy()
    b[: 2 * QK] *= SCALE
    bias = np.ascontiguousarray(b.reshape(6, 128).T)

    wtr = np.zeros((128, 2 * 768), np.float32)
    for i in range(2):
        wtr[:, i * 768:(i + 1) * 768] = w[:, i * 128:(i + 1) * 128].T

    pf = np.asarray(pos_feats, np.float32)          # [256, 9]
    posb = np.zeros((128, 72), np.float32)          # [c2, (h, row36)]
    for c2 in range(128):
        n_loc = c2 // 32
        for dl in range(NKK):
            posb[c2, 0 * 36 + n_loc * 9 + dl] = pf[c2, dl]
            posb[c2, 1 * 36 + n_loc * 9 + dl] = pf[128 + c2, dl]

    redb = np.zeros((128, NKK * 36), np.float32)
    for dl in range(NKK):
        for c2 in range(128):
            redb[c2, dl * 36 + (c2 // 32) * 9 + dl] = 1.0

    emat2 = np.zeros((100, NKK * 128), np.float32)
    for h in range(2):
        for r in range(36):
            n_loc, dl = r // 9, r % 9
            emat2[h * 64 + r,
                  dl * 128 + n_loc * 32: dl * 128 + (n_loc + 1) * 32] = 1.0

    idb = np.eye(128, dtype=np.float32)

    return (wtr.astype(np.float32), bias, posb.astype(bf), redb.astype(bf),
            emat2.astype(bf), idb.astype(bf))


def make_in_maps(x, w_proj, b_proj, pos_feats):
    x = np.asarray(x, np.float32)
    (wtr, bias, posb, redb, emat2, idb) = _host_consts(
        np.asarray(w_proj, np.float32),
        np.asarray(b_proj, np.float32),
        np.asarray(pos_feats, np.float32),
    )
    in_maps = []
    for s in range(NCORES):
        b_i, half = s // 2, s % 2
        xs = np.zeros((CIN, HS, W), np.float32)
        h0 = half * HOWN - 1
        lo, hi = max(h0, 0), min(h0 + HS, H)
        xs[:, lo - h0:hi - h0, :] = x[b_i, :, lo:hi, :]
        edge = np.ones((128, 2), np.float32)
        if half == 0:
            edge[:, 0] = 0.0
        if half == 1:
            edge[:, 1] = 0.0
        in_maps.append({
            "x": np.ascontiguousarray(xs.reshape(CIN, PXU)),
            "wtr": wtr, "bias": bias, "posb": posb, "redb": redb,
            "emat2": emat2, "idb": idb, "edge": edge,
        })
    return in_maps


def kernel(x, w_proj, b_proj, pos_feats):
    from concourse.bass_utils import run_bass_kernel_spmd

    if "nc" not in _CACHE:
        _CACHE["nc"] = _build_nc()
    nc = _CACHE["nc"]
    in_maps = make_in_maps(x, w_proj, b_proj, pos_feats)
    res = run_bass_kernel_spmd(nc, in_maps, list(range(NCORES)))
    out = np.zeros((B, OUT, H, W), np.float32)
    for s in range(NCORES):
        b_i, half = s // 2, s % 2
        out[b_i, :, half * HOWN:(half + 1) * HOWN, :] = (
            res.results[s]["o"].reshape(OUT, HOWN, W)
        )
    return out


# revision 39
# speedup vs baseline: 1.0461x; 1.0192x over previous
"""Trainium2 Bass kernel for nn_Attn2d (3x3 local window attention, 8 heads).

v3: transposed QK-reduce — the qk products are fed through the matmul
STATIONARY port (free in the cost model) with a small [128, 36] reduce
matrix as the moving operand, so each offset's logit reduction costs 36
columns instead of 512. Logits land pixel-major ([128px, 288] psum), so
softmax (exp / z-sum / reciprocal / normalize) runs on ACT+DVE along the
free dim with no PE involvement, and normalization happens BEFORE the AV
stage (po psum holds the final output, copied out directly). A PE
transpose brings normalized attention back to channel-major [100, 512]
for the expand matmuls. All-bf16 (no fp8/DoubleRow).

Layout notes: per-chunk logits psum plT [128, 288] = (h:2, blk:4, n:4,
dl:9); e_t [100, 512] with halves at partitions 0..35 / 64..99 (PE
outputs must start at partition 0/32/64/96)."""
import numpy as np

import concourse.mybir as mybir
import concourse.tile as tile
from concourse import bacc
from concourse.ap import AP

F32 = mybir.dt.float32
F32R = mybir.dt.float32r
BF16 = mybir.dt.bfloat16
AF = mybir.ActivationFunctionType
AX = mybir.AxisListType

B, CIN, H, W = 4, 256, 64, 64
QK = 256
OUT = 256
NH = 8
KW = 3
D = QK // NH
SCALE = float(D) ** (-0.25)
NCORES = 8

HOWN = H // 2
HS = HOWN + 2
WP = W + 4
C0 = 2
PXP = HS * WP
PXU = HS * W
OWNPX = HOWN * W
NKK = KW * KW

CHUNK = 512
NCHUNK = OWNPX // CHUNK
NBLK = CHUNK // 128

PCW = [256, 256, 512, 512, 384, 256]
PCO = [0, 256, 512, 1024, 1536, 1920]

SLOTS = [(0, 1), (2, 3), (4, 5), (6, 7)]
DL8 = 8

# ---- engine config (LP-balanced: ACT=proj+cb+ob, Pool=QK, DVE=AV) ----
# QK product slots (s, h): 'D' DVE, 'P' Pool (all bf16)
QK_CFG = {(0, 0): 'D', (0, 1): 'P',
          (1, 0): 'P', (1, 1): 'P',
          (2, 0): 'P', (2, 1): 'P',
          (3, 0): 'P', (3, 1): 'P'}
QK8_CFG = {0: 'P', 1: 'P'}
# AV product units (s, h): 'D' DVE direct from psum,
# 'AP' ACT-copy psum->bf16 then Pool multiply
AV_CFG = {(0, 0): 'D', (0, 1): 'D',
          (1, 0): 'D', (1, 1): 'D',
          (2, 0): 'D', (2, 1): 'AP',
          (3, 0): 'D', (3, 1): 'AP'}
AV8_CFG = {0: 'D', 1: 'AP'}


def _off(dl):
    return (dl // 3) * WP + 1 + (dl % 3)


def _build_nc():
    nc = bacc.Bacc()

    xin = nc.declare_dram_parameter("x", [CIN, PXU], F32R, isOutput=False)
    wtrd = nc.declare_dram_parameter("wtr", [128, 2 * 768], F32R, isOutput=False)
    biasd = nc.declare_dram_parameter("bias", [128, 6], F32, isOutput=False)
    posbd = nc.declare_dram_parameter("posb", [128, 72], BF16, isOutput=False)
    redbd = nc.declare_dram_parameter("redb", [128, NKK * 36], BF16, isOutput=False)
    ematd = nc.declare_dram_parameter("emat2", [100, NKK * 128], BF16, isOutput=False)
    idbd = nc.declare_dram_parameter("idb", [128, 128], BF16, isOutput=False)
    edged = nc.declare_dram_parameter("edge", [128, 2], F32, isOutput=False)
    outd = nc.declare_dram_parameter("o", [OUT, OWNPX], F32, isOutput=True)

    with tile.TileContext(nc) as tc:
        with (
            tc.tile_pool(name="const", bufs=1) as constp,
            tc.tile_pool(name="data", bufs=1) as datap,
            tc.tile_pool(name="qkp", bufs=8) as qkp,
            tc.tile_pool(name="p2p", bufs=8) as p2p,
            tc.tile_pool(name="ebp", bufs=6) as ebp,
            tc.tile_pool(name="psl", bufs=1, space="PSUM") as psl,
            tc.tile_pool(name="pse", bufs=3, space="PSUM") as pse,
            tc.tile_pool(name="pso", bufs=1, space="PSUM") as pso,
        ):
            # PE warmup (p-state ramp)
            wz = constp.tile([1, 2], BF16, tag="wz", name="wz")
            nc.gpsimd.memset(wz[:], 0.0)
            pwz = pse.tile([1, 2], F32, tag="pe", name="pwz")
            nc.tensor.matmul(pwz[:, 0:1], wz[:, 0:1], wz[:, 1:2],
                             start=True, stop=True, skip_group_check=True)

            # ---- loads (queues ordered by first use) ----
            xf = datap.tile([128, 2 * PXU], F32R, tag="xf", name="xf")
            xfv = xf[:].rearrange("p (t x) -> p t x", t=2)

            def xrhs(pci, t):
                co, cw = PCO[pci], PCW[pci]
                return xfv[:, t, co:co + cw]

            def xdma(eng, pci, t):
                co, cw = PCO[pci], PCW[pci]
                eng.dma_start(xfv[:, t, co:co + cw],
                              xin[t * 128:(t + 1) * 128, co:co + cw])

            xdma(nc.sync, 0, 0)
            xdma(nc.sync, 0, 1)
            wt8 = constp.tile([128, 2 * 768], F32R, tag="wt8", name="wt8")
            wt8v = wt8[:].rearrange("p (two m) -> p two m", two=2)
            wtdv = wtrd[:].rearrange("p (two m) -> p two m", two=2)
            for g in range(3):
                nc.gpsimd.dma_start(wt8v[:, :, g * 256:(g + 1) * 256],
                                    wtdv[:, :, g * 256:(g + 1) * 256])
            bias_t = constp.tile([128, 6], F32, tag="bias", name="bias")
            nc.scalar.dma_start(bias_t[:], biasd[:])
            xdma(nc.sync, 1, 0)
            xdma(nc.sync, 1, 1)
            posb = constp.tile([128, 72], BF16, tag="posb", name="posb")
            nc.gpsimd.dma_start(posb[:], posbd[:])
            redb = constp.tile([128, NKK * 36], BF16, tag="redb", name="redb")
            nc.gpsimd.dma_start(redb[:], redbd[:])
            xdma(nc.sync, 2, 0)
            xdma(nc.scalar, 2, 1)
            idb = constp.tile([128, 128], BF16, tag="idb", name="idb")
            nc.gpsimd.dma_start(idb[:], idbd[:])
            emat2 = constp.tile([100, NKK * 128], BF16, tag="emat2", name="emat2")
            nc.gpsimd.dma_start(emat2[:], ematd[:])
            edge_t = constp.tile([128, 2], F32, tag="edge", name="edge")
            nc.scalar.dma_start(edge_t[:], edged[:])
            xdma(nc.sync, 3, 0)
            xdma(nc.sync, 3, 1)
            xdma(nc.sync, 4, 0)
            xdma(nc.sync, 4, 1)
            xdma(nc.sync, 5, 0)
            xdma(nc.sync, 5, 1)

            q2 = datap.tile([128, 2 * PXP], BF16, tag="q2", name="q2")
            k2 = datap.tile([128, 2 * PXP], BF16, tag="k2", name="k2")
            v2 = datap.tile([128, 2 * PXP], BF16, tag="v2", name="v2")

            def padv(tl, t, r0, nr, c0, ncol):
                return (tl[:].rearrange("p (t r c) -> p t r c", t=2, c=WP)
                        [:, t, r0:r0 + nr, c0:c0 + ncol])

            for tl in (k2, v2):
                pv = tl[:].rearrange("p (t r c) -> p t r c", t=2, c=WP)
                nc.gpsimd.memset(pv[:, :, :, 0:C0], 0.0)
                nc.gpsimd.memset(pv[:, :, :, WP - C0:WP], 0.0)

            # per-chunk-parity softmax tiles
            eP = [datap.tile([128, 288], BF16, tag=f"eP{i}", name=f"eP{i}")
                  for i in range(2)]
            aP = [datap.tile([128, 288], BF16, tag=f"aP{i}", name=f"aP{i}")
                  for i in range(2)]
            zt = [datap.tile([128, 32], F32, tag=f"zt{i}", name=f"zt{i}")
                  for i in range(2)]
            rz = [datap.tile([128, 32], F32, tag=f"rz{i}", name=f"rz{i}")
                  for i in range(2)]
            et = [datap.tile([100, 512], BF16, tag=f"et{i}", name=f"et{i}")
                  for i in range(2)]

            # logits psum, double-buffered; e_t transpose staging (bf16 psum)
            plT = [psl.tile([128, 288], F32, tag=f"plT{i}", name=f"plT{i}")
                   for i in range(2)]
            etP = psl.tile([100, 512], BF16, tag="etP", name="etP")
            # rows 36..63 of etP are never written by the transposes but are
            # covered by the [100, 512] copy to et -> zero them once
            nc.vector.memset(etP[36:64, :], 0.0)

            # ---- projection (fp32r; x consumed via bitcast, no copy) ----
            def gen_proj(pci):
                for mb0 in range(0, 6, 2):
                    emit_proj(pci, mb0, mb0 + 2)
                    yield

            def emit_proj(pci, mb_lo=0, mb_hi=6):
                co, cw = PCO[pci], PCW[pci]
                r0, nr = co // W, cw // W
                for mb in range(mb_lo, mb_hi):
                    pp = pse.tile([128, cw], F32, tag="pe", name=f"pp{pci}_{mb}")
                    for i in range(2):
                        lhsT = wt8[:][:, i * 768 + mb * 128: i * 768 + (mb + 1) * 128]
                        nc.tensor.matmul(
                            pp[:], lhsT, xrhs(pci, i),
                            start=(i == 0), stop=(i == 1),
                            skip_group_check=True)
                    grp, t = mb // 2, mb % 2
                    dst = (q2, k2, v2)[grp]
                    nc.scalar.activation(padv(dst, t, r0, nr, C0, W), pp[:],
                                         AF.Identity, bias=bias_t[:, mb:mb + 1])

            def emit_edge(row, col):
                for tl in (k2, v2):
                    pv = tl[:].rearrange("p (t r c) -> p t r c", t=2, c=WP)
                    nc.gpsimd.tensor_scalar_mul(pv[:, :, row, :], pv[:, :, row, :],
                                                edge_t[:, col:col + 1])

            # ---- views ----
            def kv_pair(tl, h, ci, dla, dlb):
                base = h * PXP + (8 * ci) * WP + _off(dla)
                delta = _off(dlb) - _off(dla)
                return AP(tensor=tl[:].tensor, offset=tl[:].offset + base,
                          ap=[[2 * PXP, 128], [delta, 2], [WP, 8], [1, W]])

            def kv_single(tl, h, ci, dl):
                base = h * PXP + (8 * ci) * WP + _off(dl)
                return AP(tensor=tl[:].tensor, offset=tl[:].offset + base,
                          ap=[[2 * PXP, 128], [WP, 8], [1, W]])

            def q_own(h, ci):
                return padv(q2, h, 1 + 8 * ci, 8, C0, W)

            def q_dup(h, ci):
                return q_own(h, ci).unsqueeze(1).broadcast_to([128, 2, 8, W])

            def q_row(h, ci, r):
                # [128, W] = one image row of q (2D AP for the stationary port)
                return padv(q2, h, 1 + 8 * ci + r, 1, C0, W).squeeze(1)

            def pl_slice(ci, h, b):
                return plT[ci % 2][:, h * 144 + b * 36: h * 144 + (b + 1) * 36]

            def emit_reduce(ci, h, qk_ap, dl, stop):
                # qk_ap: [128, CHUNK] bf16 product (flat view); consumed as
                # stationary in 128-px blocks; moving = redb column block.
                for b in range(NBLK):
                    nc.tensor.matmul(
                        pl_slice(ci, h, b),
                        qk_ap[:, b * 128:(b + 1) * 128],
                        redb[:, dl * 36:(dl + 1) * 36],
                        start=False, stop=stop, skip_group_check=True)

            def gen_red(ci):
                """QK products (DVE/Pool) + transposed reduce (PE tiny MMs)."""
                for h in range(2):
                    for b in range(NBLK):
                        c0 = h * 144 + b * 36
                        for r in range(2):
                            # stationary must be a 2D AP -> one 64-px image
                            # row per pos matmul, output at partition 0/64
                            nc.tensor.matmul(
                                plT[ci % 2][r * 64:(r + 1) * 64, c0:c0 + 36],
                                q_row(h, ci, 2 * b + r),
                                posb[:, h * 36:(h + 1) * 36],
                                start=True, stop=False, skip_group_check=True)
                yield
                for s, (dla, dlb) in enumerate(SLOTS):
                    for h in range(2):
                        eng = nc.gpsimd if QK_CFG[(s, h)] == 'P' else nc.vector
                        qk = qkp.tile([128, 2 * CHUNK], BF16, tag="qk",
                                      name=f"qk{ci}_{s}_{h}")
                        qkv = qk[:].rearrange("p (two r c) -> p two r c", two=2, c=W)
                        with nc.allow_low_precision(reason="qk products"):
                            eng.tensor_mul(qkv, q_dup(h, ci),
                                           kv_pair(k2, h, ci, dla, dlb))
                        qkf = qk[:].rearrange("p (two x) -> p two x", two=2)
                        emit_reduce(ci, h, qkf[:, 0], dla, stop=False)
                        emit_reduce(ci, h, qkf[:, 1], dlb, stop=False)
                    yield
                for h in range(2):
                    eng = nc.gpsimd if QK8_CFG[h] == 'P' else nc.vector
                    qs = qkp.tile([128, CHUNK], BF16, tag="qks",
                                  name=f"qks{ci}_{h}")
                    qsv = qs[:].rearrange("p (r c) -> p r c", c=W)
                    with nc.allow_low_precision(reason="qk products"):
                        eng.tensor_mul(qsv, q_own(h, ci), kv_single(k2, h, ci, DL8))
                    emit_reduce(ci, h, qs[:], DL8, stop=True)
                yield

            def gen_av(ci):
                par = ci % 2
                # softmax: exp -> z -> 1/z -> normalize (pixel-major)
                nc.scalar.activation(eP[par][:], plT[par][:], AF.Exp)
                ev = eP[par][:].rearrange("p (h b n d) -> p h b n d",
                                          h=2, b=NBLK, n=4)
                ztv = zt[par][:].rearrange("p (h b n) -> p h b n", h=2, b=NBLK)
                nc.vector.tensor_reduce(ztv, ev, AX.X, mybir.AluOpType.add)
                nc.vector.reciprocal(rz[par][:], zt[par][:])
                rzv = (rz[par][:].rearrange("p (h b n) -> p h b n", h=2, b=NBLK)
                       .unsqueeze(4).broadcast_to([128, 2, NBLK, 4, NKK]))
                av = aP[par][:].rearrange("p (h b n d) -> p h b n d",
                                          h=2, b=NBLK, n=4)
                with nc.allow_low_precision(reason="normalized attention"):
                    nc.gpsimd.tensor_mul(av, ev, rzv)
                # transpose back to channel-major [100, 512] (halves at 0/64)
                for h in range(2):
                    for b in range(NBLK):
                        nc.tensor.transpose(
                            etP[h * 64: h * 64 + 36, b * 128:(b + 1) * 128],
                            aP[par][:, h * 144 + b * 36: h * 144 + (b + 1) * 36],
                            idb[:])
                nc.vector.tensor_copy(et[par][:], etP[:])
                yield

                def ematv(h, dl):
                    return emat2[h * 64: h * 64 + 36, dl * 128:(dl + 1) * 128]

                def etv(h):
                    return et[par][h * 64: h * 64 + 36, :]

                po = [pso.tile([128, CHUNK], F32, tag=f"po{h}", name=f"po{ci}_{h}")
                      for h in range(2)]
                started = [False, False]

                # h-major: finish half 0 early so its output copy + DMA
                # overlap half 1's units (shrinks the kernel tail)
                units = [(s, h) for h in range(2) for s in range(5)]

                def emit_expands(u):
                    s, h = units[u]
                    if s < 4:
                        dla, dlb = SLOTS[s]
                        ea = pse.tile([128, CHUNK], F32, tag="pe",
                                      name=f"ea{ci}_{s}_{h}")
                        nc.tensor.matmul(ea[:], ematv(h, dla), etv(h),
                                         start=True, stop=True,
                                         skip_group_check=True)
                        eb = pse.tile([128, CHUNK], F32, tag="pe",
                                      name=f"eb{ci}_{s}_{h}")
                        nc.tensor.matmul(eb[:], ematv(h, dlb), etv(h),
                                         start=True, stop=True,
                                         skip_group_check=True)
                        return (ea, eb)
                    ea = pse.tile([128, CHUNK], F32, tag="pe", name=f"ea8{ci}_{h}")
                    nc.tensor.matmul(ea[:], ematv(h, DL8), etv(h),
                                     start=True, stop=True, skip_group_check=True)
                    return (ea,)

                def av_product(dst, src_psum, h, dl, cfg, tagsuffix):
                    if cfg == 'D':
                        with nc.allow_low_precision(reason="av products"):
                            nc.vector.tensor_mul(
                                dst, src_psum[:].rearrange("p (r c) -> p r c", c=W),
                                kv_single(v2, h, ci, dl))
                    else:  # 'AP'
                        cb = ebp.tile([128, CHUNK], BF16, tag="ebf",
                                      name=f"cb{tagsuffix}")
                        nc.scalar.copy(cb[:], src_psum[:])
                        with nc.allow_low_precision(reason="av products"):
                            nc.gpsimd.tensor_mul(
                                dst, cb[:].rearrange("p (r c) -> p r c", c=W),
                                kv_single(v2, h, ci, dl))

                def emit_unit(u, exp_tiles):
                    s, h = units[u]
                    if s < 4:
                        dla, dlb = SLOTS[s]
                        ea, eb = exp_tiles
                        cfg = AV_CFG[(s, h)]
                        p2 = p2p.tile([128, 2 * CHUNK], BF16, tag="p2",
                                      name=f"p2{ci}_{s}_{h}")
                        p2v = p2[:].rearrange("p (two r c) -> p two r c",
                                              two=2, c=W)
                        av_product(p2v[:, 0], ea, h, dla, cfg, f"{ci}_{s}_{h}_a")
                        av_product(p2v[:, 1], eb, h, dlb, cfg, f"{ci}_{s}_{h}_b")
                        for j in range(2):
                            nc.tensor.matmul(
                                po[h][:], idb[:],
                                p2[:].rearrange("p (two x) -> p two x", two=2)[:, j, :],
                                start=not started[h], stop=False,
                                skip_group_check=True)
                            started[h] = True
                    else:
                        (ea,) = exp_tiles
                        cfg = AV8_CFG[h]
                        ps_ = p2p.tile([128, CHUNK], BF16, tag="p2s",
                                       name=f"p2s{ci}_{h}")
                        av_product(ps_[:].rearrange("p (r c) -> p r c", c=W),
                                   ea, h, DL8, cfg, f"8{ci}_{h}")
                        nc.tensor.matmul(po[h][:], idb[:], ps_[:],
                                         start=False, stop=True,
                                         skip_group_check=True)

                def emit_out(h):
                    # output copy + DMA (already normalized)
                    ob = datap.tile([128, CHUNK], F32, tag=f"ob{h}", name=f"ob{ci}_{h}")
                    nc.scalar.copy(ob[:], po[h][:])
                    nc.sync.dma_start(
                        outd[h * 128:(h + 1) * 128, ci * CHUNK:(ci + 1) * CHUNK],
                        ob[:])

                nxt = emit_expands(0)
                yield
                for u in range(len(units)):
                    cur = nxt
                    if u + 1 < len(units):
                        nxt = emit_expands(u + 1)
                    emit_unit(u, cur)
                    if units[u][0] == 4:
                        emit_out(units[u][1])
                    yield

            def drive(gens, headstart=2):
                live = list(gens)
                for _ in range(headstart):
                    if live:
                        try:
                            next(live[0])
                        except StopIteration:
                            live.pop(0)
                while live:
                    nxt = []
                    for g in live:
                        try:
                            next(g)
                            nxt.append(g)
                        except StopIteration:
                            pass
                    live = nxt

            # ---- schedule ----
            emit_proj(0)
            emit_proj(1)
            emit_proj(2)
            emit_edge(0, 0)
            drive([gen_red(0)])
            emit_proj(3)
            drive([gen_av(0), gen_red(1), gen_proj(4)], headstart=2)
            drive([gen_av(1), gen_red(2), gen_proj(5)], headstart=2)
            emit_edge(HS - 1, 1)
            drive([gen_av(2), gen_red(3)], headstart=2)
            drive([gen_av(3)])

    nc.finalize()
    return nc


_CACHE = {}


def _host_consts(w_proj, b_proj, pos_feats):
    import ml_dtypes
    bf = ml_dtypes.bfloat16

    w = np.asarray(w_proj, np.float32).copy()       # [768, 256]
    w[: 2 * QK] *= SCALE
    b = np.asarray(b_proj, np.float32).copy()
    b[: 2 * QK] *= SCALE
    bias = np.ascontiguousarray(b.reshape(6, 128).T)

    wtr = np.zeros((128, 2 * 768), np.float32)
    for i in range(2):
        wtr[:, i * 768:(i + 1) * 768] = w[:, i * 128:(i + 1) * 128].T

    pf = np.asarray(pos_feats, np.float32)          # [256, 9]
    posb = np.zeros((128, 72), np.float32)          # [c2, (h, row36)]
    for c2 in range(128):
        n_loc = c2 // 32
        for dl in range(NKK):
            posb[c2, 0 * 36 + n_loc * 9 + dl] = pf[c2, dl]
            posb[c2, 1 * 36 + n_loc * 9 + dl] = pf[128 + c2, dl]

    redb = np.zeros((128, NKK * 36), np.float32)
    for dl in range(NKK):
        for c2 in range(128):
            redb[c2, dl * 36 + (c2 // 32) * 9 + dl] = 1.0

    emat2 = np.zeros((100, NKK * 128), np.float32)
    for h in range(2):
        for r in range(36):
            n_loc, dl = r // 9, r % 9
            emat2[h * 64 + r,
                  dl * 128 + n_loc * 32: dl * 128 + (n_loc + 1) * 32] = 1.0

    idb = np.eye(128, dtype=np.float32)

    return (wtr.astype(np.float32), bias, posb.astype(bf), redb.astype(bf),
            emat2.astype(bf), idb.astype(bf))


def make_in_maps(x, w_proj, b_proj, pos_feats):
    x = np.asarray(x, np.float32)
    (wtr, bias, posb, redb, emat2, idb) = _host_consts(
        np.asarray(w_proj, np.float32),
        np.asarray(b_proj, np.float32),
        np.asarray(pos_feats, np.float32),
    )
    in_maps = []
    for s in range(NCORES):
        b_i, half = s // 2, s % 2
        xs = np.zeros((CIN, HS, W), np.float32)
        h0 = half * HOWN - 1
        lo, hi = max(h0, 0), min(h0 + HS, H)
        xs[:, lo - h0:hi - h0, :] = x[b_i, :, lo:hi, :]
        edge = np.ones((128, 2), np.float32)
        if half == 0:
            edge[:, 0] = 0.0
        if half == 1:
            edge[:, 1] = 0.0
        in_maps.append({
            "x": np.ascontiguousarray(xs.reshape(CIN, PXU)),
            "wtr": wtr, "bias": bias, "posb": posb, "redb": redb,
            "emat2": emat2, "idb": idb, "edge": edge,
        })
    return in_maps


def kernel(x, w_proj, b_proj, pos_feats):
    from concourse.bass_utils import run_bass_kernel_spmd

    if "nc" not in _CACHE:
        _CACHE["nc"] = _build_nc()
    nc = _CACHE["nc"]
    in_maps = make_in_maps(x, w_proj, b_proj, pos_feats)
    res = run_bass_kernel_spmd(nc, in_maps, list(range(NCORES)))
    out = np.zeros((B, OUT, H, W), np.float32)
    for s in range(NCORES):
        b_i, half = s // 2, s % 2
        out[b_i, :, half * HOWN:(half + 1) * HOWN, :] = (
            res.results[s]["o"].reshape(OUT, HOWN, W)
        )
    return out
